# revision 1
# baseline (speedup 1.0000x reference)
"""Trainium2 8-core kernel for RMSNorm -> QKV -> RoPE -> causal SDPA -> out-proj.

Sharding: core c = b*4 + g handles batch b (of 2) and heads 4g..4g+3 (of 16).
Each core computes a partial out-projection [dim, tokens]; the host sums the
4 head-group partials per batch (the tensor-parallel "unshard") and adds b_o.

All layouts on chip are feature-major ([feature, token]) so every matmul
contracts over partitions. The RMSNorm scale r (per token) is never applied
to x directly: it rides into Q via r-scaled RoPE cos/sin tables, into K via
the per-key `scale` operand of the exp activation, and into V via a
token-major tensor_scalar. Softmax uses the no-max-subtraction form (scores
are O(6)); the denominator comes free from a ones column appended to V.
RoPE rotate-half is a constant 128x128 partition-permutation matrix applied
on the TensorEngine.
"""

import os

import numpy as np
import ml_dtypes

BF16 = ml_dtypes.bfloat16

DIM = 1024
HEADS = 16
DIM_HEAD = 64
T = 2048  # tokens per batch
B = 2
HPC = 4  # heads per core
F = HPC * DIM_HEAD  # 256 per-core head width
KC = DIM // 128  # 8 contraction chunks

_NC_CACHE = {}


def _build_nc():
    import concourse.bacc as bacc
    import concourse.mybir as mybir
    import concourse.tile as tile
    from contextlib import ExitStack

    f32 = mybir.dt.float32
    bf16 = mybir.dt.bfloat16
    nc = bacc.Bacc()

    xT = nc.declare_dram_parameter("xT", [DIM, T], bf16, isOutput=False)
    wq = nc.declare_dram_parameter("wq", [DIM, F], bf16, isOutput=False)
    wk = nc.declare_dram_parameter("wk", [DIM, F], bf16, isOutput=False)
    wv = nc.declare_dram_parameter("wv", [DIM, F], bf16, isOutput=False)
    wo = nc.declare_dram_parameter("wo", [F, DIM], bf16, isOutput=False)
    cosT = nc.declare_dram_parameter("cosT", [128, T], bf16, isOutput=False)
    sinT = nc.declare_dram_parameter("sinT", [128, T], bf16, isOutput=False)
    perm = nc.declare_dram_parameter("perm", [128, 128], bf16, isOutput=False)
    masks = nc.declare_dram_parameter("masks", [128, 128], bf16, isOutput=False)
    ident = nc.declare_dram_parameter("ident", [128, 128], bf16, isOutput=False)
    out = nc.declare_dram_parameter("out", [DIM, T], bf16, isOutput=True)

    Exp = mybir.ActivationFunctionType.Exp
    Sqrt = mybir.ActivationFunctionType.Sqrt
    mult = mybir.AluOpType.mult
    add = mybir.AluOpType.add

    with ExitStack() as ctx:
        tc = ctx.enter_context(tile.TileContext(nc))
        consts = ctx.enter_context(tc.tile_pool(name="consts", bufs=1))
        persist = ctx.enter_context(tc.tile_pool(name="persist", bufs=1))
        work = ctx.enter_context(tc.tile_pool(name="work", bufs=4))
        vecs = ctx.enter_context(tc.tile_pool(name="vecs", bufs=1))

        # ---- load constants ----
        wq_sb = consts.tile([128, KC, F], bf16, tag="wq")
        wk_sb = consts.tile([128, KC, F], bf16, tag="wk")
        wv_sb = consts.tile([128, KC, F], bf16, tag="wv")
        wo_sb = consts.tile([128, 2, DIM], bf16, tag="wo")
        cos_sb = consts.tile([128, T], bf16, tag="cos")
        sin_sb = consts.tile([128, T], bf16, tag="sin")
        perm_sb = consts.tile([128, 128], bf16, tag="perm")
        mask_sb = consts.tile([128, 128], bf16, tag="mask")
        ones_col = consts.tile([128, 1], bf16, tag="onesc")
        id_sb = consts.tile([128, 128], bf16, tag="ident")
        xT_sb = persist.tile([128, KC, T], bf16, tag="xT")
        xT_r = xT.rearrange("(kc p) t -> p kc t", p=128)
        for kc in range(KC):
            nc.sync.dma_start(xT_sb[:, kc], xT_r[:, kc])
        nc.sync.dma_start(wk_sb, wk.rearrange("(kc p) f -> p kc f", p=128))
        nc.sync.dma_start(wq_sb, wq.rearrange("(kc p) f -> p kc f", p=128))
        nc.sync.dma_start(wv_sb, wv.rearrange("(kc p) f -> p kc f", p=128))
        nc.sync.dma_start(cos_sb, cosT[:, :])
        nc.sync.dma_start(sin_sb, sinT[:, :])
        nc.sync.dma_start(perm_sb, perm[:, :])
        nc.sync.dma_start(mask_sb, masks[:, :])
        nc.sync.dma_start(id_sb, ident[:, :])
        nc.sync.dma_start(wo_sb, wo.rearrange("(fc p) d -> p fc d", p=128))
        nc.vector.memset(ones_col, 1.0)

        # ---- phase A: r = 1/sqrt(mean(x^2)) per token (eps=1.2e-7 dropped:
        # mean-square is O(1) on this input, far below bf16 noise) ----
        xsq_all = persist.tile([128, KC, T], bf16, tag="xsq")
        ctxAC = ExitStack()
        psAC = ctxAC.enter_context(tc.tile_pool(name="psAC", bufs=1, space="PSUM"))
        psS = ctxAC.enter_context(tc.tile_pool(name="psS", bufs=2, space="PSUM"))
        psq = ctxAC.enter_context(tc.tile_pool(name="psq", bufs=3, space="PSUM"))
        psp = ctxAC.enter_context(tc.tile_pool(name="psp", bufs=2, space="PSUM"))
        if True:
            for kc in range(KC):
                nc.vector.tensor_mul(xsq_all[:, kc], xT_sb[:, kc], xT_sb[:, kc])
            ss_sb = vecs.tile([1, T], f32, tag="sssb")
            for s in range(4):
                ss_ps = psS.tile([1, 512], f32, tag="ss", name=f"ss_{s}")
                for kc in range(KC):
                    nc.tensor.matmul(
                        ss_ps,
                        lhsT=ones_col,
                        rhs=xsq_all[:, kc, s * 512 : (s + 1) * 512],
                        start=(kc == 0),
                        stop=(kc == KC - 1),
                    )
                nc.scalar.copy(out=ss_sb[:, s * 512 : (s + 1) * 512], in_=ss_ps)
            sq = vecs.tile([1, T], f32, tag="sq")
            nc.scalar.activation(sq, ss_sb, Sqrt, scale=1.0 / DIM)
            r_sb = vecs.tile([1, T], f32, tag="r")
            nc.vector.reciprocal(r_sb, sq)
            r_bf = vecs.tile([1, T], bf16, tag="rbf")
            nc.scalar.copy(out=r_bf, in_=r_sb)
            # r broadcast across partitions (gpsimd)
            r_bc = persist.tile([128, T], bf16, tag="rbc")
            nc.gpsimd.partition_broadcast(r_bc, r_bf)
            # token-major r (for V scaling and exp scale): diagonal extraction
            # r_tok[p, tt] = r_bc[p, tt*128+p] via identity-mask + free reduce
            r_tok = persist.tile([128, 16], f32, tag="rtok")
            for tt in range(16):
                dg = work.tile([128, 128], f32, tag="diag")
                nc.vector.tensor_mul(
                    dg, r_bc[:, tt * 128 : (tt + 1) * 128], id_sb
                )
                nc.vector.reduce_sum(
                    r_tok[:, tt : tt + 1], dg, axis=mybir.AxisListType.X
                )

        # ---- phase B: fold r into the Q-side rope tables ----
        cosr_sb = persist.tile([128, T], bf16, tag="cosr")
        sinr_sb = persist.tile([128, T], bf16, tag="sinr")
        nc.vector.tensor_mul(cosr_sb, cos_sb, r_bc)
        nc.vector.tensor_mul(sinr_sb, sin_sb, r_bc)

        # ---- phase C: QKV projections + RoPE ----
        # qk_sb tiles: 0=q(h0,h1) 1=q(h2,h3) 2=k(h0,h1) 3=k(h2,h3)
        qk_sb = persist.tile([128, 4, T], bf16, tag="qk")
        v_sb = persist.tile([128, 16, HPC, 65], bf16, tag="v")
        nc.vector.memset(v_sb[:, :, :, 64:65], 1.0)

        if True:
            w_of = {0: (wq_sb, 0), 1: (wq_sb, 1), 2: (wk_sb, 0), 3: (wk_sb, 1)}
            for fidx in [2, 0]:
                for tt in range(4):
                    ts = slice(tt * 512, (tt + 1) * 512)
                    wsb, fc = w_of[fidx]
                    cc = cosr_sb if fidx < 2 else cos_sb
                    ssb = sinr_sb if fidx < 2 else sin_sb
                    ps = psq.tile([128, 512], f32, tag="projqk", name=f"qk_{tt}_{fidx}")
                    for kc in range(KC):
                        nc.tensor.matmul(
                            ps,
                            lhsT=wsb[:, kc, fc * 128 : (fc + 1) * 128],
                            rhs=xT_sb[:, kc, ts],
                            start=(kc == 0),
                            stop=(kc == KC - 1),
                        )
                    raw = work.tile([128, 512], bf16, tag="qraw")
                    nc.scalar.copy(out=raw, in_=ps)
                    pp = psp.tile([128, 512], f32, tag="permps", name=f"pp_{tt}_{fidx}")
                    nc.tensor.matmul(pp, lhsT=perm_sb, rhs=raw, start=True, stop=True)
                    t1 = work.tile([128, 512], bf16, tag="ropet1")
                    nc.vector.tensor_tensor(t1, pp, ssb[:, ts], mult)
                    t2 = work.tile([128, 512], bf16, tag="ropet2")
                    nc.vector.tensor_tensor(t2, raw, cc[:, ts], mult)
                    nc.gpsimd.tensor_tensor(qk_sb[:, fidx, ts], t2, t1, add)
            for tt in range(16):
                psv = psAC.tile([128, 256], f32, tag="projv", name=f"v_{tt}")
                for kc in range(KC):
                    nc.tensor.matmul(
                        psv,
                        lhsT=xT_sb[:, kc, tt * 128 : (tt + 1) * 128],
                        rhs=wv_sb[:, kc, :],
                        start=(kc == 0),
                        stop=(kc == KC - 1),
                    )
                nc.scalar.activation(
                    out=v_sb[:, tt, :, 0:64],
                    in_=psv.rearrange("p (h d) -> p h d", h=HPC),
                    func=mybir.ActivationFunctionType.Copy,
                    scale=r_tok[:, tt : tt + 1],
                )

            for fidx in [3, 1]:
                for tt in range(4):
                    ts = slice(tt * 512, (tt + 1) * 512)
                    wsb, fc = w_of[fidx]
                    cc = cosr_sb if fidx < 2 else cos_sb
                    ssb = sinr_sb if fidx < 2 else sin_sb
                    ps = psq.tile([128, 512], f32, tag="projqk", name=f"qk2_{tt}_{fidx}")
                    for kc in range(KC):
                        nc.tensor.matmul(
                            ps,
                            lhsT=wsb[:, kc, fc * 128 : (fc + 1) * 128],
                            rhs=xT_sb[:, kc, ts],
                            start=(kc == 0),
                            stop=(kc == KC - 1),
                        )
                    raw = work.tile([128, 512], bf16, tag="qraw")
                    nc.scalar.copy(out=raw, in_=ps)
                    pp = psp.tile([128, 512], f32, tag="permps", name=f"pp2_{tt}_{fidx}")
                    nc.tensor.matmul(pp, lhsT=perm_sb, rhs=raw, start=True, stop=True)
                    t1 = work.tile([128, 512], bf16, tag="ropet1")
                    nc.vector.tensor_tensor(t1, pp, ssb[:, ts], mult)
                    t2 = work.tile([128, 512], bf16, tag="ropet2")
                    nc.vector.tensor_tensor(t2, raw, cc[:, ts], mult)
                    nc.gpsimd.tensor_tensor(qk_sb[:, fidx, ts], t2, t1, add)
        ctxAC.close()

        # ---- phase D: causal attention per head, split into q-halves so
        # scores/av PSUM tiles double-buffer within 8 banks ----
        av_all = persist.tile([128, 2, T], bf16, tag="av")
        with (
            tc.tile_pool(name="psD", bufs=2, space="PSUM") as psD,
            tc.tile_pool(name="expp", bufs=6) as expp,
        ):
            for h in range(HPC):
                qt = qk_sb[:, 0 if h < 2 else 1]
                kt = qk_sb[:, 2 if h < 2 else 3]
                rows = slice((h % 2) * 64, (h % 2) * 64 + 64)
                tidx = 0 if h < 2 else 1
                for qh in range(2):
                    qlo = qh * 1024
                    av_ps = psD.tile([65, 1024], f32, tag="av", name=f"av_{h}_{qh}")
                    nkb = 8 * (qh + 1)
                    pend = {}
                    for kb in range(nkb + 1):
                        if kb < nkb:
                            c0 = max(kb * 128 - qlo, 0)
                            s0h = c0 // 512
                            sc = psD.tile(
                                [128, 1024], f32, tag="sc", name=f"sc_{h}_{qh}_{kb}"
                            )
                            for s in range(s0h, 2):
                                nc.tensor.matmul(
                                    sc[:, s * 512 : (s + 1) * 512],
                                    lhsT=kt[rows, kb * 128 : (kb + 1) * 128],
                                    rhs=qt[rows, qlo + s * 512 : qlo + (s + 1) * 512],
                                    start=True,
                                    stop=True,
                                )
                            ex = expp.tile([128, 1024], bf16, tag="exp")
                            if c0 > s0h * 512:
                                nc.gpsimd.memset(ex[:, s0h * 512 : c0], 0.0)
                            nc.scalar.activation(
                                ex[:, c0:1024],
                                sc[:, c0:1024],
                                Exp,
                                scale=r_tok[:, kb : kb + 1],
                            )
                            if kb * 128 >= qlo:
                                nc.vector.tensor_tensor(
                                    ex[:, c0 : c0 + 128],
                                    ex[:, c0 : c0 + 128],
                                    mask_sb,
                                    mult,
                                )
                            pend[kb] = (ex, s0h)
                        if kb >= 1:
                            exp_prev, sp = pend.pop(kb - 1)
                            for s in range(sp, 2):
                                nc.tensor.matmul(
                                    av_ps[:, s * 512 : (s + 1) * 512],
                                    lhsT=v_sb[:, kb - 1, h, :],
                                    rhs=exp_prev[:, s * 512 : (s + 1) * 512],
                                    start=(kb - 1 == 0),
                                    stop=(kb - 1 == 4 * (2 * qh + s) + 3),
                                )
                    rec = vecs.tile([1, 1024], f32, tag="rec", name=f"rec_{h}_{qh}")
                    nc.vector.reciprocal(rec, av_ps[64:65, :])
                    rb_sb = vecs.tile([64, 1024], f32, tag="recbc", name=f"rb_{h}_{qh}")
                    nc.gpsimd.partition_broadcast(rb_sb, rec)
                    nc.vector.tensor_tensor(
                        av_all[rows, tidx, qlo : qlo + 1024],
                        av_ps[0:64],
                        rb_sb,
                        mult,
                    )

        # ---- phase E: out projection (partial; host sums groups) ----
        with tc.tile_pool(name="psE", bufs=4, space="PSUM") as psE:
            for do in range(8):
                for tt in range(4):
                    ts = slice(tt * 512, (tt + 1) * 512)
                    po = psE.tile([128, 512], f32, tag="out", name=f"o_{do}_{tt}")
                    for fc in range(2):
                        nc.tensor.matmul(
                            po,
                            lhsT=wo_sb[:, fc, do * 128 : (do + 1) * 128],
                            rhs=av_all[:, fc, ts],
                            start=(fc == 0),
                            stop=(fc == 1),
                        )
                    ob = work.tile([128, 512], bf16, tag="ob")
                    if (do + tt) % 2 == 0:
                        nc.scalar.copy(out=ob, in_=po)
                    else:
                        nc.vector.tensor_copy(out=ob, in_=po)
                    nc.sync.dma_start(
                        out.rearrange("(do p) t -> p do t", p=128)[:, do, ts], ob
                    )
    nc.compile()
    return nc


def _host_inputs(x, norm_w, w_qkv, w_o, sin, cos):
    """Build the 8 per-core input maps (all bf16)."""
    n = T
    w_eff = np.asarray(w_qkv, np.float64) * np.asarray(norm_w, np.float64)[:, None]
    sin_n = np.asarray(sin, np.float32)[:n]  # [T, 64]
    cos_n = np.asarray(cos, np.float32)[:n]
    sign = np.concatenate([-np.ones(32, np.float32), np.ones(32, np.float32)])
    cos_tile = np.tile(cos_n.T, (2, 1))  # [128, T]
    sin_tile = np.tile((sin_n * sign[None, :]).T, (2, 1))  # [128, T]
    perm = np.zeros((128, 128), np.float32)
    for m in range(128):
        d = m % 64
        k = m + 32 if d < 32 else m - 32
        perm[k, m] = 1.0
    ident_np = np.eye(128, dtype=np.float32)
    sel_np = np.zeros((128, 128), np.float32)
    sel_np[0, :] = 1.0
    ql = np.arange(128)[None, :]
    key = np.arange(128)[:, None]
    masks = (ql >= key).astype(np.float32)

    in_maps = []
    for c in range(8):
        b, g = c // 4, c % 4
        fs = slice(g * F, (g + 1) * F)
        in_maps.append(
            {
                "xT": np.ascontiguousarray(np.asarray(x, np.float32)[b].T).astype(BF16),
                "wq": (w_eff[:, 0:DIM][:, fs] * (DIM_HEAD ** -0.5)).astype(BF16),
                "wk": w_eff[:, DIM : 2 * DIM][:, fs].astype(BF16),
                "wv": w_eff[:, 2 * DIM : 3 * DIM][:, fs].astype(BF16),
                "wo": np.asarray(w_o, np.float32)[fs, :].astype(BF16),
                "cosT": cos_tile.astype(BF16),
                "sinT": sin_tile.astype(BF16),
                "perm": perm.astype(BF16),
                "masks": masks.astype(BF16),
                "ident": ident_np.astype(BF16),
            }
        )
    return in_maps


def kernel(x, norm_w, w_qkv, w_o, b_o, sin, cos):
    from concourse.bass_utils import run_bass_kernel_spmd

    if "nc" not in _NC_CACHE:
        _NC_CACHE["nc"] = _build_nc()
    nc = _NC_CACHE["nc"]
    in_maps = _host_inputs(x, norm_w, w_qkv, w_o, sin, cos)
    trace = bool(int(os.environ.get("KERNEL_TRACE", "0")))
    res = run_bass_kernel_spmd(nc, in_maps, core_ids=list(range(8)), trace=trace)
    if trace and res.exec_time_ns is not None:
        print(f"HW exec time: {res.exec_time_ns} ns")
    outs = [r["out"].astype(np.float32) for r in res.results]  # [1024, T] fm
    b_o = np.asarray(b_o, np.float32)
    full = np.empty((B, T, DIM), np.float32)
    for b in range(B):
        acc = outs[b * 4] + outs[b * 4 + 1] + outs[b * 4 + 2] + outs[b * 4 + 3]
        full[b] = acc.T + b_o[None, :]
    return full



# revision 33
# speedup vs baseline: 1.2459x; 1.2459x over previous
"""Trainium2 8-core kernel for RMSNorm -> QKV -> RoPE -> causal SDPA -> out-proj.

Sharding: core c = b*4 + g handles batch b (of 2) and heads 4g..4g+3 (of 16).
Each core computes a partial out-projection [dim, tokens]; the host sums the
4 head-group partials per batch and adds b_o.

Key layout/scheduling choices (driven by the TimelineSim cost model):
- QKV projections run as fp8e4 DoubleRow matmuls (256-deep contraction at
  0.5 cycles/col) on host-prepared hi/lo splits of x and the weights; the
  dropped lo*lo term is ~2^-8 relative. Scales (SW=64 on weights, SX=4 on x,
  keeping hi/lo out of fp8 denormals) unfold in the psum->sbuf copies.
- Attention score/AV matmuls and exp activations use exact causal column
  ranges (c0 = kb*128 - qlo) instead of 512-aligned chunks.
- RMSNorm r = exp(-0.5*ln(mean(x^2))): ln/exp/copy/square live in one ACT
  function set -> single table load. Token-major r_tok comes from 1-column
  matmuls (xsq.T @ ones), nearly free in PE cycles.
- K projections (which never need r) lead each 512-token chunk so PE work
  starts as soon as the serialized DMA stream lands each chunk.
- Phase D runs qh-major; the out-projection for the first token half is
  emitted between the qh0 and qh1 groups, with its psum borrowed from the
  rotating "sc" score tiles (no extra PSUM banks, PE queue never drains).
- Out-proj copies rotate over ACT/DVE/Pool; outputs stage in SBUF and leave
  in [128,4,512] DMAs.
"""

import os

import numpy as np
import ml_dtypes

BF16 = ml_dtypes.bfloat16
F8 = ml_dtypes.float8_e4m3

DIM = 1024
HEADS = 16
DIM_HEAD = 64
T = 2048  # tokens per batch
B = 2
HPC = 4  # heads per core
F = HPC * DIM_HEAD  # 256 per-core head width
KC = DIM // 128  # 8 contraction chunks
SW = 64.0  # host-side weight scale (keeps fp8 hi/lo out of denormals)
SX = 4.0  # host-side activation scale; SW*SX folds out at psum->sbuf copies

_NC_CACHE = {}


def _build_nc():
    import concourse.bacc as bacc
    import concourse.mybir as mybir
    import concourse.tile as tile
    from contextlib import ExitStack

    f32 = mybir.dt.float32
    bf16 = mybir.dt.bfloat16
    fp8 = mybir.dt.float8e4
    nc = bacc.Bacc()

    xhi = nc.declare_dram_parameter("xhi", [128, KC, T], fp8, isOutput=False)
    xlo = nc.declare_dram_parameter("xlo", [128, KC, T], fp8, isOutput=False)
    wqhi = nc.declare_dram_parameter("wqhi", [128, KC, F], fp8, isOutput=False)
    wqlo = nc.declare_dram_parameter("wqlo", [128, KC, F], fp8, isOutput=False)
    wkhi = nc.declare_dram_parameter("wkhi", [128, KC, F], fp8, isOutput=False)
    wklo = nc.declare_dram_parameter("wklo", [128, KC, F], fp8, isOutput=False)
    wvhi = nc.declare_dram_parameter("wvhi", [128, KC, F], fp8, isOutput=False)
    wvlo = nc.declare_dram_parameter("wvlo", [128, KC, F], fp8, isOutput=False)
    wo = nc.declare_dram_parameter("wo", [128, 2, DIM], bf16, isOutput=False)
    cosT = nc.declare_dram_parameter("cosT", [128, T], bf16, isOutput=False)
    sinT = nc.declare_dram_parameter("sinT", [128, T], bf16, isOutput=False)
    perm = nc.declare_dram_parameter("perm", [128, 128], bf16, isOutput=False)
    negtri = nc.declare_dram_parameter("negtri", [128, 128], bf16, isOutput=False)
    ident = nc.declare_dram_parameter("ident", [128, 128], bf16, isOutput=False)
    out = nc.declare_dram_parameter("out", [128, KC, T], bf16, isOutput=True)

    A = mybir.ActivationFunctionType
    DR = mybir.MatmulPerfMode.DoubleRow
    mult = mybir.AluOpType.mult
    add = mybir.AluOpType.add

    with ExitStack() as ctx:
        tc = ctx.enter_context(tile.TileContext(nc))
        consts = ctx.enter_context(tc.tile_pool(name="consts", bufs=1))
        persist = ctx.enter_context(tc.tile_pool(name="persist", bufs=1))
        work = ctx.enter_context(tc.tile_pool(name="work", bufs=4))
        vecs = ctx.enter_context(tc.tile_pool(name="vecs", bufs=1))
        vecs2 = ctx.enter_context(tc.tile_pool(name="vecs2", bufs=2))

        # ---- SBUF tiles ----
        wk_hi = consts.tile([128, KC, F], fp8, tag="wkhi")
        wk_lo = consts.tile([128, KC, F], fp8, tag="wklo")
        wq_hi = consts.tile([128, KC, F], fp8, tag="wqhi")
        wq_lo = consts.tile([128, KC, F], fp8, tag="wqlo")
        wv_hi = consts.tile([128, KC, F], fp8, tag="wvhi")
        wv_lo = consts.tile([128, KC, F], fp8, tag="wvlo")
        wo_sb = consts.tile([128, 2, DIM], bf16, tag="wo")
        cos_sb = consts.tile([128, T], bf16, tag="cos")
        sin_sb = consts.tile([128, T], bf16, tag="sin")
        perm_sb = consts.tile([128, 128], bf16, tag="perm")
        ngt_sb = consts.tile([128, 128], bf16, tag="negtri")
        id_sb = consts.tile([128, 128], bf16, tag="ident")
        ones_col = consts.tile([128, 1], bf16, tag="onesc")
        xh_sb = persist.tile([128, KC, T], fp8, tag="xhi")
        xl_sb = persist.tile([128, KC, T], fp8, tag="xlo")
        cosr_sb = persist.tile([128, T], bf16, tag="cosr")
        sinr_sb = persist.tile([128, T], bf16, tag="sinr")
        r_bc = persist.tile([128, T], bf16, tag="rbc")
        r_tok = persist.tile([128, 16], f32, tag="rtok")
        rtok16 = persist.tile([128, 16], f32, tag="rtok16")
        qk_sb = persist.tile([128, 4, T], bf16, tag="qk")
        v_sb = persist.tile([128, 16, HPC, 65], bf16, tag="v")
        av_all = persist.tile([128, 2, T], bf16, tag="av")

        nc.vector.memset(ones_col, 1.0)
        nc.vector.memset(v_sb[:, :, :, 64:65], 1.0)

        # ---- DMA stream, ordered so k-projection of chunk s can start the
        # moment its x lands ----
        nc.sync.dma_start(wk_hi, wkhi[:, :, :])
        nc.sync.dma_start(xh_sb[:, :, 0:512], xhi[:, :, 0:512])
        nc.sync.dma_start(wk_lo, wklo[:, :, :])
        nc.sync.dma_start(xl_sb[:, :, 0:512], xlo[:, :, 0:512])
        nc.sync.dma_start(perm_sb, perm[:, :])
        for s in range(1, 4):
            ts = slice(s * 512, (s + 1) * 512)
            nc.sync.dma_start(xh_sb[:, :, ts], xhi[:, :, ts])
            nc.sync.dma_start(xl_sb[:, :, ts], xlo[:, :, ts])
        for s in range(4):
            ts = slice(s * 512, (s + 1) * 512)
            nc.sync.dma_start(cos_sb[:, ts], cosT[:, ts])
            nc.sync.dma_start(sin_sb[:, ts], sinT[:, ts])
        nc.sync.dma_start(wq_hi, wqhi[:, :, :])
        nc.sync.dma_start(wq_lo, wqlo[:, :, :])
        nc.sync.dma_start(wv_hi, wvhi[:, :, :])
        nc.sync.dma_start(wv_lo, wvlo[:, :, :])
        nc.sync.dma_start(ngt_sb, negtri[:, :])
        nc.sync.dma_start(id_sb, ident[:, :])
        nc.sync.dma_start(wo_sb, wo[:, :, :])

        ctxAC = ExitStack()
        psS = ctxAC.enter_context(tc.tile_pool(name="psS", bufs=1, space="PSUM"))
        psq = ctxAC.enter_context(tc.tile_pool(name="psq", bufs=2, space="PSUM"))
        psp = ctxAC.enter_context(tc.tile_pool(name="psp", bufs=1, space="PSUM"))
        psv = ctxAC.enter_context(tc.tile_pool(name="psv", bufs=2, space="PSUM"))
        psR = ctxAC.enter_context(tc.tile_pool(name="psR", bufs=2, space="PSUM"))

        # ---- phase A per 512-chunk: xsq -> mst -> r_tok; the row-major r
        # comes back from r_tok via a partition-gather DMA ----
        # (eps=1.2e-7 dropped: mean-square is O(1), far below bf16 noise)
        def phaseA(s):
            ts = slice(s * 512, (s + 1) * 512)
            sl4 = slice(s * 4, s * 4 + 4)
            xsq = work.tile([128, KC, 512], bf16, tag="xsq", name=f"xsq_{s}")
            for kc in range(KC):
                if kc < 3:
                    nc.vector.tensor_mul(xsq[:, kc], xh_sb[:, kc, ts], xh_sb[:, kc, ts])
                elif kc < 6:
                    nc.scalar.activation(xsq[:, kc], xh_sb[:, kc, ts], A.Square)
                else:
                    nc.gpsimd.tensor_mul(xsq[:, kc], xh_sb[:, kc, ts], xh_sb[:, kc, ts])
            # token-major mean-square via 1-col matmuls (xsq.T @ ones)
            mst = psS.tile([128, 4], f32, tag="mst", name=f"mst_{s}")
            for tt in range(4):
                for kc in range(KC):
                    nc.tensor.matmul(
                        mst[:, tt : tt + 1],
                        lhsT=xsq[:, kc, tt * 128 : (tt + 1) * 128],
                        rhs=ones_col[:, 0:1],
                        start=(kc == 0),
                        stop=(kc == KC - 1),
                    )
            # r = rsqrt(mean xsq): Abs_reciprocal_sqrt shares its ACT table
            # set with square/copy, so only attention's Exp loads a new table
            nc.scalar.activation(
                r_tok[:, sl4], mst, A.Abs_reciprocal_sqrt,
                scale=1.0 / (DIM * SX * SX),
            )
            nc.vector.tensor_scalar_mul(
                rtok16[:, sl4], r_tok[:, sl4], 1.0 / (SW * SX)
            )
            # row-major r = transpose(r_tok) on PE, rows hopped to partition
            # 0 via relative-partition ACT copies, then one broadcast
            rtbf = work.tile([128, 4], bf16, tag="rtbf", name=f"rtbf_{s}")
            nc.vector.tensor_copy(out=rtbf, in_=r_tok[:, sl4])
            rrow = vecs2.tile([1, 512], bf16, tag="rrow", name=f"rrow_{s}")
            for tt in range(4):
                rtr = psR.tile([1, 128], bf16, tag="rtr", name=f"rtr_{s}_{tt}")
                nc.tensor.matmul(
                    rtr, lhsT=rtbf[:, tt : tt + 1], rhs=id_sb, is_transpose=True
                )
                nc.scalar.copy(out=rrow[:, tt * 128 : (tt + 1) * 128], in_=rtr)
            nc.gpsimd.partition_broadcast(r_bc[:, ts], rrow)
            nc.vector.tensor_mul(cosr_sb[:, ts], cos_sb[:, ts], r_bc[:, ts])
            nc.vector.tensor_mul(sinr_sb[:, ts], sin_sb[:, ts], r_bc[:, ts])

        # ---- phase C building blocks ----
        def proj_qk(fidx, s, whi_sb, wlo_sb, fc, cc, ssb, ps=None, pp=None):
            """12 DoubleRow matmuls (hi*hi first) -> scaled copy -> rope."""
            ts = slice(s * 512, (s + 1) * 512)
            ms = slice(fc * 128, fc * 128 + 128)
            if ps is None:
                ps = psq.tile([128, 512], f32, tag="projqk", name=f"qk_{fidx}_{s}")
            for j in range(4):
                kk = slice(2 * j, 2 * j + 2)
                nc.tensor.matmul(ps, lhsT=whi_sb[:, kk, ms], rhs=xh_sb[:, kk, ts],
                                 start=(j == 0), stop=False, perf_mode=DR)
            for j in range(4):
                kk = slice(2 * j, 2 * j + 2)
                nc.tensor.matmul(ps, lhsT=wlo_sb[:, kk, ms], rhs=xh_sb[:, kk, ts],
                                 start=False, stop=False, perf_mode=DR)
            for j in range(4):
                kk = slice(2 * j, 2 * j + 2)
                nc.tensor.matmul(ps, lhsT=whi_sb[:, kk, ms], rhs=xl_sb[:, kk, ts],
                                 start=False, stop=(j == 3), perf_mode=DR)
            raw = work.tile([128, 512], bf16, tag="qraw")
            nc.scalar.activation(raw, ps, A.Copy, scale=1.0 / (SW * SX))
            if pp is None:
                pp = psp.tile([128, 512], f32, tag="permps", name=f"pp_{fidx}_{s}")
            nc.tensor.matmul(pp, lhsT=perm_sb, rhs=raw, start=True, stop=True)
            t1 = work.tile([128, 512], bf16, tag="ropet1")
            nc.vector.tensor_tensor(t1, pp, ssb[:, ts], mult)
            t2 = work.tile([128, 512], bf16, tag="ropet2")
            nc.vector.tensor_tensor(t2, raw, cc[:, ts], mult)
            nc.gpsimd.tensor_tensor(qk_sb[:, fidx, ts], t2, t1, add)

        def proj_v(tt, ps):
            """V for all 4 heads at token chunk tt; scale r_tok/(SW*SX)."""
            tsl = slice(tt * 128, (tt + 1) * 128)
            for j in range(4):
                kk = slice(2 * j, 2 * j + 2)
                nc.tensor.matmul(ps, lhsT=xh_sb[:, kk, tsl], rhs=wv_hi[:, kk, :],
                                 start=(j == 0), stop=False, perf_mode=DR)
            for j in range(4):
                kk = slice(2 * j, 2 * j + 2)
                nc.tensor.matmul(ps, lhsT=xl_sb[:, kk, tsl], rhs=wv_hi[:, kk, :],
                                 start=False, stop=False, perf_mode=DR)
            for j in range(4):
                kk = slice(2 * j, 2 * j + 2)
                nc.tensor.matmul(ps, lhsT=xh_sb[:, kk, tsl], rhs=wv_lo[:, kk, :],
                                 start=False, stop=(j == 3), perf_mode=DR)
            nc.vector.tensor_scalar_mul(
                v_sb[:, tt, :, 0:64],
                ps.rearrange("p (h d) -> p h d", h=HPC),
                rtok16[:, tt : tt + 1],
            )

        # K projections lead (no dependency on r); phase A trails per chunk.
        for s in range(4):
            proj_qk(2, s, wk_hi, wk_lo, 0, cos_sb, sin_sb)
            proj_qk(3, s, wk_hi, wk_lo, 1, cos_sb, sin_sb)
            phaseA(s)
        for s in range(4):
            proj_qk(0, s, wq_hi, wq_lo, 0, cosr_sb, sinr_sb)
        for s in range(4):
            proj_qk(1, s, wq_hi, wq_lo, 1, cosr_sb, sinr_sb)
        for tt in range(8):
            proj_v(tt, psv.tile([128, 256], f32, tag="projv", name=f"v_{tt}"))
        ctxAC.close()

        # ---- phases D+E interleaved: attention qh-major; out-projection for
        # token half qh emitted right after its qh group, psum borrowed from
        # the rotating "sc" tiles ----
        with (
            tc.tile_pool(name="psD", bufs=2, space="PSUM") as psD,
            tc.tile_pool(name="expp", bufs=8) as expp,
            tc.tile_pool(name="obuf", bufs=2) as obufp,
        ):

            def attn_pair(ha, hb, qh, fill=None):
                """Two heads of a tile-pair interleaved through one kb loop:
                each exp gets ~2 PE ops of shadow, hiding sem latency, with
                the same sc/av PSUM footprint. `fill[kb]` emits extra work
                (V projections / out-proj groups) inside the loop. The
                first-half normalize runs mid-loop, as soon as columns
                [0:512] have taken their last AV accumulation."""
                fill = fill or {}
                qlo = qh * 1024
                nkb = 8 * (qh + 1)
                half_kb = 3 + 8 * qh  # last kb whose AV touches cols [0:512)
                cfg = {}
                av_ps = {}
                for h in (ha, hb):
                    cfg[h] = (
                        qk_sb[:, 0 if h < 2 else 1],
                        qk_sb[:, 2 if h < 2 else 3],
                        slice((h % 2) * 64, (h % 2) * 64 + 64),
                        0 if h < 2 else 1,
                    )
                    av_ps[h] = psD.tile(
                        [128, 1024], f32, tag="av", name=f"av_{h}_{qh}"
                    )

                def normalize(h, hf):
                    _, _, rows, tidx = cfg[h]
                    cs = slice(hf * 512, hf * 512 + 512)
                    rec = vecs2.tile([1, 512], f32, tag="rec",
                                     name=f"rec_{h}_{qh}_{hf}")
                    nc.vector.reciprocal(rec, av_ps[h][64:65, cs])
                    rb = vecs2.tile([64, 512], f32, tag="recbc",
                                    name=f"rb_{h}_{qh}_{hf}")
                    nc.gpsimd.partition_broadcast(rb, rec)
                    nc.vector.tensor_tensor(
                        av_all[rows, tidx,
                               qlo + hf * 512 : qlo + hf * 512 + 512],
                        av_ps[h][0:64, cs],
                        rb,
                        mult,
                    )

                pend = {}
                for kb in range(nkb + 1):
                    if kb < nkb:
                        c0 = max(kb * 128 - qlo, 0)
                        diag = kb * 128 >= qlo
                        for h in (ha, hb):
                            qt, kt, rows, _ = cfg[h]
                            sc = psD.tile(
                                [128, 1024], f32, tag="sc",
                                name=f"sc_{h}_{qh}_{kb}",
                            )
                            # matmul output must stay within one 2KB PSUM
                            # bank: split at column 512
                            for lo, hi in ((c0, 512), (max(c0, 512), 1024)):
                                if lo >= hi:
                                    continue
                                in_diag = diag and lo <= c0 < hi
                                nc.tensor.matmul(
                                    sc[:, lo:hi],
                                    lhsT=kt[rows, kb * 128 : (kb + 1) * 128],
                                    rhs=qt[rows, qlo + lo : qlo + hi],
                                    start=True,
                                    stop=not in_diag,
                                    skip_group_check=True,
                                )
                                if in_diag:
                                    # -1e9 upper triangle on the diagonal
                                    # block; exp then yields exact zeros
                                    nc.tensor.matmul(
                                        sc[:, c0 : c0 + 128],
                                        lhsT=id_sb,
                                        rhs=ngt_sb,
                                        start=False,
                                        stop=True,
                                        skip_group_check=True,
                                    )
                            ex = expp.tile([128, 1024], bf16, tag="exp")
                            nc.scalar.activation(
                                ex[:, c0:1024],
                                sc[:, c0:1024],
                                A.Exp,
                                scale=r_tok[:, kb : kb + 1],
                            )
                            pend[(h, kb)] = (ex, c0)
                    if kb >= 1:
                        for h in (ha, hb):
                            exp_prev, cp = pend.pop((h, kb - 1))
                            for lo, hi in ((cp, 512), (max(cp, 512), 1024)):
                                if lo >= hi:
                                    continue
                                nc.tensor.matmul(
                                    av_ps[h][0:65, lo:hi],
                                    lhsT=v_sb[:, kb - 1, h, :],
                                    rhs=exp_prev[:, lo:hi],
                                    start=(kb - 1 == 0),
                                    stop=(kb - 1 == (half_kb if hi == 512
                                                     else nkb - 1)),
                                    skip_group_check=True,
                                )
                        if kb - 1 == half_kb:
                            normalize(ha, 0)
                            normalize(hb, 0)
                    if kb in fill:
                        fill[kb]()
                normalize(ha, 1)
                normalize(hb, 1)

            obtiles = {}

            def po_pair(tt, pr, use_act, tag="sc"):
                """2 out-proj columns (tt, do = 2*pr, 2*pr+1) on the two
                512-col halves of one borrowed psum tile."""
                ts = slice(tt * 512, (tt + 1) * 512)
                if tt not in obtiles:
                    obtiles[tt] = obufp.tile(
                        [128, KC, 512], bf16, tag="ob", name=f"ob_{tt}"
                    )
                ob = obtiles[tt]
                pot = psD.tile([128, 1024], f32, tag=tag, name=f"po_{tt}_{pr}")
                for half in range(2):
                    do = pr * 2 + half
                    po = pot[:, half * 512 : half * 512 + 512]
                    for fc in range(2):
                        nc.tensor.matmul(
                            po,
                            lhsT=wo_sb[:, fc, do * 128 : (do + 1) * 128],
                            rhs=av_all[:, fc, ts],
                            start=(fc == 0),
                            stop=(fc == 1),
                        )
                    eng = (do + tt) % (3 if use_act else 2)
                    if use_act and eng == 2:
                        nc.scalar.copy(out=ob[:, do], in_=po)
                    elif eng == 1:
                        nc.gpsimd.tensor_copy(out=ob[:, do], in_=po)
                    else:
                        nc.vector.tensor_copy(out=ob[:, do], in_=po)
                if pr == 1:
                    nc.sync.dma_start(out[:, 0:4, ts], ob[:, 0:4])
                elif pr == 3:
                    nc.sync.dma_start(out[:, 4:8, ts], ob[:, 4:8])

            def v_fill(tt):
                def go():
                    pot = psD.tile([128, 1024], f32, tag="sc", name=f"vps_{tt}")
                    proj_v(tt, pot[:, 0:256])
                return go

            def po_fill(tt, pr, use_act=False):
                def go():
                    po_pair(tt, pr, use_act)
                return go

            def q_fill(fidx, s, whi_sb, wlo_sb, fc, cc, ssb):
                def go():
                    pot = psD.tile([128, 1024], f32, tag="sc",
                                   name=f"qf_{fidx}_{s}")
                    proj_qk(fidx, s, whi_sb, wlo_sb, fc, cc, ssb,
                            ps=pot[:, 0:512], pp=pot[:, 512:1024])
                return go

            # attention with deferrable work threaded through the kb loops:
            # V 8..15 and the tt0/1/2 out-projections spread over the pairs;
            # only the tt3 out-projection remains as tail
            attn_pair(0, 1, 0, fill={5: v_fill(8), 6: v_fill(9),
                                     7: v_fill(10), 8: v_fill(11)})
            attn_pair(2, 3, 0, fill={5: v_fill(12), 6: v_fill(13),
                                     7: v_fill(14), 8: v_fill(15)})
            attn_pair(0, 1, 1, fill={11: po_fill(0, 0), 13: po_fill(0, 1),
                                     15: po_fill(0, 2), 16: po_fill(0, 3)})
            attn_pair(2, 3, 1, fill={
                11: po_fill(1, 0), 12: po_fill(1, 1),
                13: po_fill(1, 2), 14: po_fill(1, 3),
                15: po_fill(2, 0), 16: po_fill(2, 1)})
            po_pair(2, 2, use_act=True)
            po_pair(2, 3, use_act=True)
            # tail: last token quarter out-projection (borrow both tags)
            for pr in range(4):
                po_pair(3, pr, use_act=True, tag="sc" if pr % 2 == 0 else "av")
    nc.compile()
    return nc


def _hi_lo(a):
    hi = a.astype(F8)
    lo = (a - hi.astype(np.float32)).astype(F8)
    return hi, lo


def _pack_pkf(a):
    """[1024, n] -> [128, 8, n] with row = kc*128 + p."""
    n = a.shape[1]
    return np.ascontiguousarray(a.reshape(KC, 128, n).transpose(1, 0, 2))


def _host_inputs(x, norm_w, w_qkv, w_o, sin, cos):
    """Build the 8 per-core input maps."""
    n = T
    w_eff = np.asarray(w_qkv, np.float64) * np.asarray(norm_w, np.float64)[:, None]
    w_eff = w_eff.astype(np.float32)
    sin_n = np.asarray(sin, np.float32)[:n]  # [T, 64]
    cos_n = np.asarray(cos, np.float32)[:n]
    sign = np.concatenate([-np.ones(32, np.float32), np.ones(32, np.float32)])
    cos_tile = np.tile(cos_n.T, (2, 1))  # [128, T]
    sin_tile = np.tile((sin_n * sign[None, :]).T, (2, 1))  # [128, T]
    perm = np.zeros((128, 128), np.float32)
    for m in range(128):
        d = m % 64
        k = m + 32 if d < 32 else m - 32
        perm[k, m] = 1.0
    ql = np.arange(128)[None, :]
    key = np.arange(128)[:, None]
    negtri = np.where(ql >= key, 0.0, -1e9).astype(np.float32)
    ident = np.eye(128, dtype=np.float32)

    in_maps = []
    for c in range(8):
        b, g = c // 4, c % 4
        fs = slice(g * F, (g + 1) * F)
        xT = np.ascontiguousarray(np.asarray(x, np.float32)[b].T) * SX  # [1024, T]
        xhi, xlo = _hi_lo(_pack_pkf(xT))
        wq = w_eff[:, 0:DIM][:, fs] * (SW * DIM_HEAD**-0.5)
        wk = w_eff[:, DIM : 2 * DIM][:, fs] * SW
        wv = w_eff[:, 2 * DIM : 3 * DIM][:, fs] * SW
        wqhi, wqlo = _hi_lo(_pack_pkf(wq))
        wkhi, wklo = _hi_lo(_pack_pkf(wk))
        wvhi, wvlo = _hi_lo(_pack_pkf(wv))
        wo = np.asarray(w_o, np.float32)[fs, :]  # [256, 1024]
        wo_p = np.ascontiguousarray(wo.reshape(2, 128, DIM).transpose(1, 0, 2))
        in_maps.append(
            {
                "xhi": xhi, "xlo": xlo,
                "wqhi": wqhi, "wqlo": wqlo,
                "wkhi": wkhi, "wklo": wklo,
                "wvhi": wvhi, "wvlo": wvlo,
                "wo": wo_p.astype(BF16),
                "cosT": cos_tile.astype(BF16),
                "sinT": sin_tile.astype(BF16),
                "perm": perm.astype(BF16),
                "negtri": negtri.astype(BF16),
                "ident": ident.astype(BF16),
            }
        )
    return in_maps


def kernel(x, norm_w, w_qkv, w_o, b_o, sin, cos):
    from concourse.bass_utils import run_bass_kernel_spmd

    if "nc" not in _NC_CACHE:
        _NC_CACHE["nc"] = _build_nc()
    nc = _NC_CACHE["nc"]
    in_maps = _host_inputs(x, norm_w, w_qkv, w_o, sin, cos)
    trace = bool(int(os.environ.get("KERNEL_TRACE", "0")))
    res = run_bass_kernel_spmd(nc, in_maps, core_ids=list(range(8)), trace=trace)
    if trace and res.exec_time_ns is not None:
        print(f"HW exec time: {res.exec_time_ns} ns")
    b_o = np.asarray(b_o, np.float32)
    full = np.empty((B, T, DIM), np.float32)
    outs = []
    for r in res.results:
        o = r["out"].astype(np.float32)  # [128, 8, T]
        outs.append(o.transpose(1, 0, 2).reshape(DIM, T))
    for b in range(B):
        acc = outs[b * 4] + outs[b * 4 + 1] + outs[b * 4 + 2] + outs[b * 4 + 3]
        full[b] = acc.T + b_o[None, :]
    return full
